# revision 31
# baseline (speedup 1.0000x reference)
"""Trainium2 Bass kernel for AtomFeaturizer (embedding_lookup, 8 cores).

Strategy: the whole featurizer is one K=98 contraction per atom against a
fused table T [98, 128] bf16:
  - 75 rows: one-hot of the six categorical indices (tables concatenated,
    bias b folded into the E_atom rows)
  - 20 rows: one-hot of the four bond-count slots over counts 0..4; each row
    carries E_bond[c] (zeroed for c==0 -> the mask) PLUS the linear
    bond-count term (c/4)*W[3+j] folded in
  - 3 rows: scalar3 (x W[0:3])
One-hot features are built on-device: a small "broadcast" matmul (S matrix)
replicates each atom's index value across its section's partitions, then one
DVE tensor_scalar(is_equal) against a per-partition iota produces the one-hot
block.

Table-stationary layout: the main matmul keeps T stationary in the PE array
and streams the per-atom feature columns as the moving operand, so the result
lands in PSUM as [D=128 partitions, atoms]. The f32->bf16 cast (ScalarE copy)
and the output DMA keep that transposed layout: DRAM output is out[128,
125000] bf16 per core, contiguous per partition; the host transposes and
upcasts to the final [N, 128] f32 (bf16 rounding is ~2e-3 of the output
scale, well inside the 2e-2 gate). Benefits vs an atom-stationary layout: no
per-128-atom stationary reload, ~4x fewer instructions, all unit-stride APs,
and half the HBM write traffic.

Data parallel over atoms: 125000 per NeuronCore (no padding), 25 blocks of
5000 atoms; per block one SWDGE input DMA pair (13 bf16 rows of packed
features), per 1000 atoms one HWDGE output DMA piece alternating between the
SP and ACT queues. PSUM is a single 4-deep pool of [128, 1000] f32 tiles:
the broadcast result occupies rows 0..94 until the is_equal consumes it,
then the main matmul overwrites the same banks (start=True) with the output.
Per 1000-atom group: 2 bcast matmuls + 1 is_equal + 2 main matmuls + 1
ScalarE copy; the DVE is_equal (PSUM-f32 source, 1x mode) is the pipeline
bottleneck at ~1.17ns/atom.
"""
import numpy as np
import ml_dtypes
from contextlib import ExitStack

from concourse import bacc, mybir
import concourse.bass as bass
import concourse.tile as tile
from concourse.bass_utils import run_bass_kernel_spmd

BF16 = ml_dtypes.bfloat16
NCORES = 8
N_TOTAL = 1_000_000
D = 128

SEC_BASES = [0, 46, 52, 63, 66, 71]
SEC_SIZES = [46, 6, 11, 3, 5, 4]
K_OH = 95     # 75 categorical one-hot rows + 20 bond one-hot rows
K_MAIN = 98   # + 3 scalar3 rows (the linear bond term is folded into the
              # bond one-hot rows of T: T[75+5j+c] += (c/4) * W[3+j])

GROUP = 1000   # atoms per group: one [128, group] f32 psum tile (2 banks)
HALF = 512     # max atoms per matmul (one psum bank of f32)
BLOCK = 5000   # atoms per DMA block
N_CORE = 125000  # 1M atoms / 8 cores, no padding

_NC_CACHE = {}


def build_consts(E_atom, E_deg, E_chg, E_hyb, E_h, E_chi, E_bond, W, b):
    T = np.zeros((K_MAIN, D), np.float32)
    T[0:46] = E_atom + b[None, :]
    T[46:52] = E_deg
    T[52:63] = E_chg
    T[63:66] = E_hyb
    T[66:71] = E_h
    T[71:75] = E_chi
    for j in range(4):
        for c in range(5):
            # bond one-hot row carries both the (masked) bond embedding and
            # the linear bond-count contribution (c/4) * W[3+j]
            T[75 + 5 * j + c] = ((E_bond[c] if c > 0 else 0.0)
                                 + (c / 4.0) * W[3 + j])
    T[95:98] = W[0:3]

    S = np.zeros((10, K_OH), np.float32)
    for t, (base, size) in enumerate(zip(SEC_BASES, SEC_SIZES)):
        S[4 + t, base:base + size] = 1.0
    for j in range(4):
        S[j, 75 + 5 * j: 75 + 5 * j + 5] = 1.0

    C = np.concatenate([np.arange(s) for s in SEC_SIZES]
                       + [np.arange(5)] * 4).astype(np.float32)
    return T.astype(BF16), S.astype(BF16), np.ascontiguousarray(C[:, None])


def build_packed(atom_idx, degree_idx, charge_idx, hybrid_idx, numh_idx,
                 chiral_idx, bond_counts, scalar3):
    # rows 0..2 scalar3 (linear feature rows -> main_tile[95:98]); rows 3..6
    # bond counts and 7..12 categorical indices (the 10 broadcast-mm operand
    # rows -> bin_tile[0:10]).
    n = atom_idx.shape[0]
    packed = np.empty((13, n), np.float32)
    packed[0:3] = scalar3.T
    packed[3:7] = bond_counts.T
    for i, idx in enumerate([atom_idx, degree_idx, charge_idx, hybrid_idx,
                             numh_idx, chiral_idx]):
        packed[7 + i] = idx
    return packed.astype(BF16)


def build_nc3(n_pad, block=BLOCK, bufs_main=5, bufs_outs=3,
              bufs_ps=4, passes=1, out_piece=1000, group=GROUP, out_eng=1,
              dbg_skip=""):
    key = ("v3", n_pad, block, bufs_main, bufs_outs, bufs_ps,
           passes, out_piece, group, out_eng, dbg_skip)
    if key in _NC_CACHE:
        return _NC_CACHE[key]
    skip = set(dbg_skip.split(",")) if dbg_skip else set()
    assert n_pad % block == 0 and block % group == 0
    nblocks = n_pad // block
    ngroups = block // group
    # per-group matmul column spans; each must fit one psum bank (512 f32)
    chunks = []
    c0 = 0
    while c0 < group:
        c1 = min(c0 + HALF, group)
        chunks.append((c0, c1))
        c0 = c1
    bf = mybir.dt.bfloat16
    f32 = mybir.dt.float32

    nc = bacc.Bacc("TRN2", target_bir_lowering=False, debug=False)
    packed_d = nc.dram_tensor("packed", [13, n_pad], bf, kind="ExternalInput")
    s_d = nc.dram_tensor("s_mat", [10, K_OH], bf, kind="ExternalInput")
    t_d = nc.dram_tensor("t_mat", [K_MAIN, D], bf, kind="ExternalInput")
    cvec_d = nc.dram_tensor("cvec", [K_OH, 1], f32, kind="ExternalInput")
    out_d = nc.dram_tensor("out", [D, n_pad], bf, kind="ExternalOutput")

    with tile.TileContext(nc) as tc, ExitStack() as ctx:
        consts = ctx.enter_context(tc.tile_pool(name="consts", bufs=1))
        bin_pool = ctx.enter_context(tc.tile_pool(name="bin", bufs=bufs_main))
        main_pool = ctx.enter_context(tc.tile_pool(name="main", bufs=bufs_main))
        outs_pool = ctx.enter_context(tc.tile_pool(name="outs", bufs=bufs_outs))
        # One PSUM pool: each group's [128, group] f32 tile first
        # holds the broadcast result in rows 0..94, which the is_equal
        # consumes; the main matmul then overwrites the same banks (start=True
        # clears has_written) with the [D, GROUP] output. 4 bufs = 8 banks,
        # giving a 4-group rotation that absorbs scheduling waves.
        ps_pool = ctx.enter_context(
            tc.tile_pool(name="ps", bufs=bufs_ps, space=bass.MemorySpace.PSUM))

        s_t = consts.tile([10, K_OH], bf)
        nc.sync.dma_start(s_t[:], s_d.ap())
        t_t = consts.tile([K_MAIN, D], bf)
        nc.sync.dma_start(t_t[:], t_d.ap())
        cvec_t = consts.tile([K_OH, 1], f32)
        nc.sync.dma_start(cvec_t[:], cvec_d.ap())

        pap = packed_d.ap()
        oap = out_d.ap()

        # Software-pipelined emission: the PE instruction stream must
        # interleave as bc(g), bc(g+1), mm(g), bc(g+2), mm(g+1), ... so the
        # in-order PE queue never has mm(g) (which waits on the DVE is_equal
        # of group g) blocking bc(g+1). `pending` holds groups whose bcast is
        # emitted but whose consume stages (is_equal, main mm, ACT copy,
        # piece-final out DMA) are deferred by one group. The output DMA is
        # chopped into out_piece-atom pieces so input DMAs can interleave on
        # the DMA engines instead of stalling behind one block-sized store.
        pending = []

        def consume():
            ps, main_t, outs_t, lo, blk_of_g = pending.pop(0)
            if "iseq" not in skip:
                nc.vector.tensor_scalar(
                    main_t[0:K_OH, lo:lo + group], ps[0:K_OH, :],
                    cvec_t[:, 0:1], None, mybir.AluOpType.is_equal)
            if "mm" not in skip:
                for h0, h1 in chunks:
                    nc.tensor.matmul(
                        ps[:, h0:h1], t_t[:, :],
                        main_t[0:K_MAIN, lo + h0:lo + h1],
                        start=True, stop=True)
            if "act" not in skip:
                nc.scalar.copy(outs_t[:, lo:lo + group], ps[:, :])
            if (lo + group) % out_piece == 0 and "out" not in skip:
                plo = lo + group - out_piece
                cols = slice(blk_of_g * block + plo,
                             blk_of_g * block + plo + out_piece)
                piece_idx = (blk_of_g * block + plo) // out_piece
                eng = (nc.scalar if (out_eng and piece_idx % 2) else nc.sync)
                eng.dma_start(oap[:, cols], outs_t[:, plo:plo + out_piece])

        total_blocks = nblocks * passes
        prefetch = bufs_main - 1
        fetched = {}  # fetch index -> (bin_t, main_t)

        def fetch(i):
            if i >= total_blocks or i in fetched:
                return
            cols = slice((i % nblocks) * block, (i % nblocks + 1) * block)
            with tc.high_priority():
                bin_t = bin_pool.tile([10, block], bf)
                nc.gpsimd.dma_start(bin_t[:], pap[3:13, cols])
                main_t = main_pool.tile([K_MAIN, block], bf)
                # scalar3 linear feature rows straight from DRAM into the
                # bottom rows of the moving operand
                nc.gpsimd.dma_start(main_t[95:98, :], pap[0:3, cols])
            fetched[i] = (bin_t, main_t)

        for i in range(prefetch):
            fetch(i)
        for bi in range(total_blocks):
            blk = bi % nblocks
            fetch(bi)
            bin_t, main_t = fetched.pop(bi)
            outs_t = outs_pool.tile([D, block], bf)
            for g in range(ngroups):
                if g == 1:
                    fetch(bi + prefetch)  # keep the input pipe `prefetch` deep
                lo = g * group
                ps = ps_pool.tile([D, group], f32)
                if "bcast" not in skip:
                    for h0, h1 in chunks:
                        nc.tensor.matmul(
                            ps[0:K_OH, h0:h1], s_t[:, :],
                            bin_t[0:10, lo + h0:lo + h1],
                            start=True, stop=True)
                pending.append((ps, main_t, outs_t, lo, blk))
                if len(pending) >= 2:
                    consume()
        while pending:
            consume()
    nc.compile()
    _NC_CACHE[key] = nc
    return nc


def _prepare(inputs):
    inputs = {k: np.asarray(v) for k, v in inputs.items()}
    T, S, C = build_consts(
        inputs['E_atom'].astype(np.float32), inputs['E_deg'].astype(np.float32),
        inputs['E_chg'].astype(np.float32), inputs['E_hyb'].astype(np.float32),
        inputs['E_h'].astype(np.float32), inputs['E_chi'].astype(np.float32),
        inputs['E_bond'].astype(np.float32), inputs['W'].astype(np.float32),
        inputs['b'].astype(np.float32))
    packed = build_packed(
        inputs['atom_idx'], inputs['degree_idx'], inputs['charge_idx'],
        inputs['hybrid_idx'], inputs['numh_idx'], inputs['chiral_idx'],
        inputs['bond_counts'], inputs['scalar3'])
    n = packed.shape[1]
    n_core = -(-n // NCORES)
    n_pad = -(-n_core // BLOCK) * BLOCK
    if n_pad * NCORES != n:
        pad = np.zeros((packed.shape[0], n_pad * NCORES - n), BF16)
        packed = np.concatenate([packed, pad], axis=1)
    in_maps = []
    for c in range(NCORES):
        p = packed[:, c * n_pad:(c + 1) * n_pad]
        in_maps.append({
            "packed": np.ascontiguousarray(p), "s_mat": S, "t_mat": T,
            "cvec": C,
        })
    return n, n_pad, in_maps


def _run(inputs, trace=False, **kw):
    n_total, n_pad, in_maps = _prepare(inputs)
    nc = build_nc3(n_pad)
    res = run_bass_kernel_spmd(nc, in_maps, list(range(NCORES)), trace=trace, **kw)
    out = np.concatenate(
        [res.results[c]["out"].T for c in range(NCORES)], axis=0)
    return out[:n_total].astype(np.float32), res


def kernel(**inputs) -> np.ndarray:
    out, _ = _run(inputs, trace=False)
    return out


# ---------------------------------------------------------------------------
# Timing harness (not used by kernel()): repeated on-device execution with
# pre-staged inputs and donated zero output buffers, mirroring
# bass2jax.run_bass_via_pjrt's shard_map build.
# ---------------------------------------------------------------------------

def _build_exec(nc, n_cores):
    import jax
    from jax.experimental.shard_map import shard_map
    from jax.sharding import Mesh, PartitionSpec
    from concourse import bass2jax

    bass2jax.install_neuronx_cc_hook()
    partition_name = (nc.partition_id_tensor.name
                      if nc.partition_id_tensor else None)
    in_names, out_names, out_avals = [], [], []
    for alloc in nc.m.functions[0].allocations:
        if not isinstance(alloc, mybir.MemoryLocationSet):
            continue
        name = alloc.memorylocations[0].name
        if alloc.kind == "ExternalInput":
            if name != partition_name:
                in_names.append(name)
        elif alloc.kind == "ExternalOutput":
            out_names.append(name)
            out_avals.append(jax.core.ShapedArray(
                tuple(alloc.tensor_shape), mybir.dt.np(alloc.dtype)))
    n_params = len(in_names)
    all_in = list(in_names + out_names)
    if partition_name is not None:
        all_in.append(partition_name)
    all_in = tuple(all_in)

    def _body(*args):
        operands = list(args)
        if partition_name is not None:
            operands.append(bass2jax.partition_id_tensor())
        outs = bass2jax._bass_exec_p.bind(
            *operands, out_avals=tuple(out_avals), in_names=all_in,
            out_names=tuple(out_names),
            lowering_input_output_aliases=(),
            sim_require_finite=True, sim_require_nnan=True, nc=nc)
        return tuple(outs)

    devices = jax.devices()[:n_cores]
    mesh = Mesh(np.asarray(devices), ("core",))
    nin = n_params + len(out_names)
    donate = tuple(range(n_params, nin))
    sharded = jax.jit(
        shard_map(_body, mesh=mesh, in_specs=(PartitionSpec("core"),) * nin,
                  out_specs=(PartitionSpec("core"),) * len(out_names),
                  check_rep=False),
        donate_argnums=donate, keep_unused=True)
    return sharded, mesh, in_names, out_names, out_avals


def time_nc(nc, in_maps, iters=16):
    import time as _time
    import jax
    from jax.sharding import NamedSharding, PartitionSpec

    sharded, mesh, in_names, out_names, out_avals = _build_exec(nc, NCORES)
    sh = NamedSharding(mesh, PartitionSpec("core"))
    gin = []
    for name in in_names:
        cat = np.concatenate([np.asarray(m[name]) for m in in_maps], axis=0)
        gin.append(jax.device_put(cat, sh))
    zero_sets = []
    for _ in range(iters + 1):
        zero_sets.append([
            jax.device_put(np.zeros((NCORES * av.shape[0], *av.shape[1:]),
                                    av.dtype), sh)
            for av in out_avals])
    r = sharded(*gin, *zero_sets[0])
    jax.block_until_ready(r)
    del r
    t0 = _time.perf_counter()
    rs = [sharded(*gin, *zero_sets[1 + i]) for i in range(iters)]
    jax.block_until_ready(rs)
    dt = _time.perf_counter() - t0
    return dt / iters * 1e9


def time_pair(nc_a, nc_b, in_maps_a, in_maps_b=None, reps=10):
    ta, tb = time_pair_raw(nc_a, nc_b, in_maps_a, in_maps_b, reps)
    ta, tb = sorted(ta), sorted(tb)
    return ta[len(ta) // 2] * 1e9, tb[len(tb) // 2] * 1e9


def time_pair_raw(nc_a, nc_b, in_maps_a, in_maps_b=None, reps=10):
    """Interleave executions of two kernels; return raw per-call second lists.

    Robust-ish to the multi-ms, drifting axon-relay dispatch overhead: the two
    kernels see the same overhead distribution, so median(b) - median(a)
    estimates the device-time difference."""
    import time as _time
    import jax
    from jax.sharding import NamedSharding, PartitionSpec

    if in_maps_b is None:
        in_maps_b = in_maps_a
    execs = []
    for nc, in_maps in ((nc_a, in_maps_a), (nc_b, in_maps_b)):
        sharded, mesh, in_names, out_names, out_avals = _build_exec(nc, NCORES)
        sh = NamedSharding(mesh, PartitionSpec("core"))
        gin = []
        for name in in_names:
            cat = np.concatenate([np.asarray(m[name]) for m in in_maps], axis=0)
            gin.append(jax.device_put(cat, sh))
        zeros = [
            jax.device_put(np.zeros((NCORES * av.shape[0], *av.shape[1:]),
                                    av.dtype), sh)
            for av in out_avals]
        execs.append((sharded, gin, zeros, out_avals, sh))

    def one_call(i):
        sharded, gin, zeros, out_avals, sh = execs[i]
        import jax as _jax
        t0 = _time.perf_counter()
        r = sharded(*gin, *zeros)
        _jax.block_until_ready(r)
        dt = _time.perf_counter() - t0
        # donation consumed the zero buffers; recycle outputs as next zeros
        execs[i] = (sharded, gin, list(r), out_avals, sh)
        return dt

    one_call(0), one_call(1)  # warmup/compile
    ta, tb = [], []
    for _ in range(reps):
        ta.append(one_call(0))
        tb.append(one_call(1))
    return ta, tb


def time_kernel(inputs, iters=16, **kw):
    n_core, n_pad, in_maps = _prepare(inputs)
    nc = build_nc3(n_pad, **kw)
    return time_nc(nc, in_maps, iters)
